# revision 6
# baseline (speedup 1.0000x reference)
"""Trainium2 Bass kernel for nn_LocalTransformerBlock (sparse sliding-window attention).

Self-contained: accepts FULL inputs, shards across 8 NeuronCores internally
(batch x sequence-quarter, with redundant halo compute; no collectives),
returns the FULL output.

Layout strategy: activations flow TRANSPOSED ([feature, token]) so every
matmul keeps weights stationary (lhsT) and streams activation columns.
Attention is computed per 512-token window; window w attends windows
{w-1, w} with exact 513-token causal masking, per head-pair with row-packed
(tile_position) K=64 matmuls. Softmax skips max-subtraction (inputs are
bounded: |sim| <= 8 unmasked, <= ~98 masked; a -30 bias keeps exp in fp32
range) and masks multiplicatively AFTER exp with 0/1 bf16 masks.
"""

import numpy as np
import ml_dtypes

import concourse.bass as bass
import concourse.mybir as mybir
import concourse.tile as tile
import concourse.bass_isa as bass_isa
from concourse import bacc
from concourse.bass_utils import run_bass_kernel_spmd

f32 = mybir.dt.float32
bf16 = mybir.dt.bfloat16
AF = mybir.ActivationFunctionType
ALU = mybir.AluOpType
bf = ml_dtypes.bfloat16

# ---------------- problem constants (hardcoded) ----------------
B, N, DIM = 2, 8192, 1024
HEADS, DH, W = 8, 64, 512
INNER = 512
HID = 2730
HIDP = 2816          # padded to 22*128
FCH = HIDP // 128    # 22
DEPTH = 2
QK_SCALE = 8.0
NCORES = 8
OWN = 2048           # tokens owned per core
T0 = 3584            # L0 token span (7 windows: 3 halo + 4 own)
SRCW = [7, 6]        # qkv windows per layer
ATTW = [(1, 7), (1, 6)]   # attention windows [lo, hi)
FFW = [(1, 7), (2, 6)]    # feed-forward windows [lo, hi)
NMASK = 11           # 6 (L0) + 5 (L1) attention windows

# per-key-chunk active query segment [qlo, qhi)
SEGLO = [0, 0, 0, 0, 0, 128, 256, 384]
SEGHI = [128, 256, 384, 512, 512, 512, 512, 512]
SEGO = [0, 128, 384, 768, 1280, 1792, 2176, 2432]  # col offset in packed mask


# ---------------- host-side table builders ----------------

def _rope_tables(s, q_scale, k_scale):
    """CA/CB tables [DEPTH][128, T0] f32 for q and k, per-core (abs position s)."""
    j = np.arange(T0)
    n = (s - 1536 + j).astype(np.float64)         # absolute token index
    r = (j % W).astype(np.float64)                # window-relative index
    d = np.arange(DH)
    inv_freq = 1.0 / (10000.0 ** ((2 * (d % 32)) / DH))
    f = n[None, :] * inv_freq[:, None]            # [64, T0]
    cosf, sinf = np.cos(f), np.sin(f)
    base = ((2 * (d % 32)) + 0.4 * DH) / (1.4 * DH)  # [64]
    scq = base[:, None] ** (r[None, :] / 256.0)   # [64, T0]
    sck = base[:, None] ** (-r[None, :] / 256.0)
    sign = np.where(d < 32, -1.0, 1.0)
    sig = (d + 32) % DH                           # sigma(d)
    out = []
    for l in range(DEPTH):
        qs, ks = q_scale[l].astype(np.float64), k_scale[l].astype(np.float64)
        caq = QK_SCALE * qs[:, None] * cosf * scq
        cbq = QK_SCALE * sign[:, None] * qs[sig][:, None] * sinf * scq
        cak = ks[:, None] * cosf * sck
        cbk = sign[:, None] * ks[sig][:, None] * sinf * sck
        out.append([np.tile(t, (2, 1)).astype(np.float32) for t in (caq, cbq, cak, cbk)])
    return out  # [l][4][128, T0]


def _masks(first_core):
    """[NMASK, 128, 2560] bf16 packed per-kb 0/1 masks."""
    m = np.zeros((NMASK, 128, 2560), np.float32)
    # which mask index is the first real window (prev fully masked)?
    # L0: attn wi 1..6 -> idx wi-1 ; first real = wi 3 -> idx 2
    # L1: attn vj 1..5 -> idx 6+(vj-1); first real = vj 2 -> idx 7
    first_idx = {2, 7} if first_core else set()
    for mi in range(NMASK):
        for kb in range(8):
            seg = np.arange(SEGLO[kb], SEGHI[kb])
            cr = np.arange(128)
            cglob = 128 * kb + cr
            if kb < 4:
                allow = cglob[:, None] >= seg[None, :]
                if mi in first_idx:
                    allow = np.zeros_like(allow)
            else:
                allow = seg[None, :] >= (cglob[:, None] - W)
            m[mi, :, SEGO[kb]:SEGO[kb] + len(seg)] = allow
    return m.astype(bf)


def _gvec():
    d = np.arange(DH)
    base = ((2 * (d % 32)) + 0.4 * DH) / (1.4 * DH)
    g = (base ** 2.0).astype(np.float32)
    return np.tile(g, 2)[:, None]  # [128, 1]


def _ones16():
    """[8, 128, 16] bf16: sqsum selector. fc 0..3 -> q heads, 4..7 -> k heads."""
    o = np.zeros((8, 128, 16), np.float32)
    r = np.arange(128)
    for fc in range(8):
        col = 2 * (fc % 4) + r // 64 + 8 * (fc // 4)
        o[fc, r, col] = 1.0
    return o.astype(bf)


# ---------------- device program ----------------

def build_nc():
    nc = bacc.Bacc("TRN2", target_bir_lowering=False, debug=False,
                   num_devices=NCORES)

    def din(name, shape, dt):
        return nc.dram_tensor(name, shape, dt, kind="ExternalInput").ap()

    xin = din("xin", [DIM, T0 + 1], f32)
    wqkv_d = din("wqkv", [DEPTH, DIM, 3 * INNER], bf16)
    wo_d = din("wo", [DEPTH, INNER, DIM], bf16)
    w1a_d = din("w1a", [DEPTH, DIM, HIDP], bf16)
    w1g_d = din("w1g", [DEPTH, DIM, HIDP], bf16)
    w2_d = din("w2", [DEPTH, HIDP, DIM], bf16)
    tabs_d = din("tabs", [DEPTH, 4, 128, T0], f32)
    masks_d = din("masks", [NMASK, 128, 2560], bf16)
    ones16_d = din("ones16", [8, 128, 16], bf16)
    gvec_d = din("gvec", [128, 1], f32)
    lng_d = din("lng", [DEPTH, 128, 8], f32)
    lnb_d = din("lnb", [DEPTH, 128, 8], f32)
    b1a_d = din("b1a", [DEPTH, 128, FCH], f32)
    b1g_d = din("b1g", [DEPTH, 128, FCH], f32)
    b2_d = din("b2", [DEPTH, 128, 8], f32)
    out_d = nc.dram_tensor("out", [DIM, OWN], f32, kind="ExternalOutput").ap()

    with tile.TileContext(nc) as tc:
        with (
            tc.tile_pool(name="const", bufs=1) as constp,
            tc.tile_pool(name="dram", bufs=1, space="DRAM") as dramp,
        ):
            x1buf = dramp.tile([DIM, 3073], f32, tag="x1")
            x2buf = dramp.tile([DIM, 3073], f32, tag="x2")

            gvec_t = constp.tile([128, 1], f32, tag="gvec")
            nc.sync.dma_start(out=gvec_t[:], in_=gvec_d[:])
            ones16_t = []
            for fc in range(8):
                t = constp.tile([128, 16], bf16, tag=f"o16_{fc}")
                nc.sync.dma_start(out=t[:], in_=ones16_d[fc])
                ones16_t.append(t)
            zcol_t = constp.tile([128, 1], f32, tag="zcol")
            nc.vector.memset(zcol_t[:], 0.0)
            c_eps24 = constp.tile([128, 1], f32, tag="c_eps24")
            nc.vector.memset(c_eps24[:], 1e-24)
            c_m30 = constp.tile([128, 1], f32, tag="c_m30")
            nc.vector.memset(c_m30[:], -30.0)
            c_eps5 = constp.tile([128, 1], f32, tag="c_eps5")
            nc.vector.memset(c_eps5[:], 1e-5)
            c_invd = constp.tile([128, 1], f32, tag="c_invd")
            nc.vector.memset(c_invd[:], 1.0 / DIM)
            consts = (c_eps24, c_m30, c_eps5, c_invd)

            for l in range(DEPTH):
                xsrc = xin if l == 0 else x1buf
                xdst = x1buf if l == 0 else None
                _layer(nc, tc, l, xsrc, xdst, x2buf, out_d,
                       wqkv_d, wo_d, w1a_d, w1g_d, w2_d, tabs_d, masks_d,
                       ones16_t, gvec_t, zcol_t, consts,
                       lng_d, lnb_d, b1a_d, b1g_d, b2_d)

    nc.compile()
    return nc


def _layer(nc, tc, l, xsrc, xdst, x2buf, out_d,
           wqkv_d, wo_d, w1a_d, w1g_d, w2_d, tabs_d, masks_d,
           ones16_t, gvec_t, zcol_t, consts, lng_d, lnb_d, b1a_d, b1g_d, b2_d):
    c_eps24, c_m30, c_eps5, c_invd = consts
    srcw = SRCW[l]
    att_lo, att_hi = ATTW[l]
    ff_lo, ff_hi = FFW[l]
    tbl_off = 0 if l == 0 else 512

    # ---------------- attention phase ----------------
    with (
        tc.tile_pool(name=f"wts{l}", bufs=1) as wtp,
        tc.tile_pool(name=f"attn{l}", bufs=1) as ap_,
        tc.tile_pool(name=f"psum{l}", bufs=1, space="PSUM") as pp,
    ):
        # weights resident
        wqkv_t = []
        for cc in range(8):
            t = wtp.tile([128, 3 * INNER], bf16, tag=f"wqkv{cc}")
            nc.sync.dma_start(out=t[:], in_=wqkv_d[l, 128 * cc:128 * cc + 128, :])
            wqkv_t.append(t)
        wo_t = []
        for pc in range(4):
            t = wtp.tile([128, DIM], bf16, tag=f"wo{pc}")
            nc.sync.dma_start(out=t[:], in_=wo_d[l, 128 * pc:128 * pc + 128, :])
            wo_t.append(t)

        # zero the shift-surrogate column of the x2 buffer
        for cc in range(8):
            nc.sync.dma_start(out=x2buf[128 * cc:128 * cc + 128, 0:1], in_=zcol_t[:])

        _pn = [0]

        def P(shape, tag, bufs):
            _pn[0] += 1
            return pp.tile(shape, f32, tag=tag, bufs=bufs, name=f"ps_{l}_{tag}_{_pn[0]}")

        prev = None  # (krot, va) of previous window
        for wi in range(srcw):
            base = 1 + W * wi
            tb = tbl_off + W * wi

            # --- load x window (fp32 for residual, bf16 for matmul rhs) ---
            xw, xb = [], []
            for cc in range(8):
                t = ap_.tile([128, 513], f32, tag="xw", bufs=8)
                nc.sync.dma_start(
                    out=t[:], in_=xsrc[128 * cc:128 * cc + 128, base - 1:base + 512])
                xw.append(t)
                tb16 = ap_.tile([128, 513], bf16, tag="xb", bufs=8)
                nc.vector.tensor_copy(tb16[:], t[:])
                xb.append(tb16)

            # --- rope tables for this window ---
            tabs = []
            for k in range(4):
                t = ap_.tile([128, 512], f32, tag="tab", bufs=5)
                nc.sync.dma_start(out=t[:], in_=tabs_d[l, k, :, tb:tb + 512])
                tabs.append(t)
            caq, cbq, cak, cbk = tabs

            # --- q/k (transposed out) + squares ---
            qraw, q2 = [], []
            ss_ps = P([16, 512], "ssacc", 1)
            for fc in range(8):
                ps = P([128, 512], "mm", 2)
                for cc in range(8):
                    sh = 1 if cc < 4 else 0
                    nc.tensor.matmul(
                        ps[:], wqkv_t[cc][:, 128 * fc:128 * fc + 128],
                        xb[cc][:, sh:sh + 512],
                        start=(cc == 0), stop=(cc == 7))
                raw = ap_.tile([128, 512], f32, tag="qraw", bufs=8)
                nc.scalar.copy(out=raw[:], in_=ps[:])
                sq = ap_.tile([128, 512], bf16, tag="q2", bufs=8)
                nc.scalar.activation(out=sq[:], in_=ps[:], func=AF.Square)
                qraw.append(raw)
                q2.append(sq)
                nc.tensor.matmul(ss_ps[:], ones16_t[fc][:], sq[:],
                                 start=(fc == 0), stop=(fc == 7))

            # --- v (natural orientation) + ones-augmented tiles ---
            va = []
            for tc_ in range(4):
                ps = P([128, 512], "mm", 2)
                for cc in range(8):
                    sh = 1 if cc < 4 else 0
                    nc.tensor.matmul(
                        ps[:], xb[cc][:, sh + 128 * tc_:sh + 128 * tc_ + 128],
                        wqkv_t[cc][:, 2 * INNER:3 * INNER],
                        start=(cc == 0), stop=(cc == 7))
                t = ap_.tile([128, 8, 65], bf16, tag="va", bufs=8)
                nc.vector.memset(t[:], 1.0)
                nc.scalar.copy(out=t[:, :, 0:64], in_=ps[:].rearrange("p (h d) -> p h d", h=8))
                va.append(t)

            # --- l2norm reciprocals ---
            ssq = ap_.tile([16, 512], f32, tag="ssq", bufs=3)
            nc.scalar.activation(out=ssq[:], in_=ss_ps[:], func=AF.Sqrt, bias=c_eps24[:16, :])
            rsq = ap_.tile([16, 512], f32, tag="rsq", bufs=3)
            nc.vector.reciprocal(out=rsq[:], in_=ssq[:])

            def bcast64(dst_ap, src_row_ap):
                src = bass.AP(tensor=src_row_ap.tensor, offset=src_row_ap.offset,
                              ap=[list(src_row_ap.ap[0]), [0, 64]] + list(src_row_ap.ap[1:]))
                nc.sync.dma_start(out=dst_ap, in_=src)

            # --- rotary + normalize (q gets xpos-recentering g on a copy) ---
            qrot, q2g, krot = [], [], []
            for fc in range(8):
                isq = fc < 4
                hc = fc % 4
                rb = ap_.tile([128, 512], f32, tag="rb", bufs=4)
                row0 = 2 * hc + (0 if isq else 8)
                bcast64(rb[0:64, :], rsq[row0:row0 + 1, :])
                bcast64(rb[64:128, :], rsq[row0 + 1:row0 + 2, :])

                rot = ap_.tile([128, 512], f32, tag="rot", bufs=4)
                for blk in range(4):
                    nc.sync.dma_start(
                        out=rot[32 * blk:32 * blk + 32, :],
                        in_=qraw[fc][32 * (blk ^ 1):32 * (blk ^ 1) + 32, :])

                ca, cb = (caq, cbq) if isq else (cak, cbk)
                t1 = ap_.tile([128, 512], f32, tag="rt", bufs=4)
                nc.vector.tensor_mul(t1[:], qraw[fc][:], ca[:])
                t2 = ap_.tile([128, 512], f32, tag="rt", bufs=4)
                nc.vector.tensor_mul(t2[:], rot[:], cb[:])
                t3 = ap_.tile([128, 512], f32, tag="rt", bufs=4)
                nc.vector.tensor_add(t3[:], t1[:], t2[:])
                rot_bf = ap_.tile([128, 512], bf16, tag="qk_bf", bufs=14)
                nc.vector.tensor_mul(rot_bf[:], t3[:], rb[:])
                if isq:
                    qrot.append(rot_bf)
                    t4 = ap_.tile([128, 512], bf16, tag="q2g", bufs=5)
                    nc.vector.tensor_scalar_mul(t4[:], rot_bf[:], gvec_t[:])
                    q2g.append(t4)
                else:
                    krot.append(rot_bf)

            # --- attention ---
            if wi >= att_lo and prev is not None:
                krot_p, va_p = prev
                midx = (wi - 1) if l == 0 else (6 + wi - 1)
                msk = ap_.tile([128, 2560], bf16, tag="msk", bufs=2)
                nc.sync.dma_start(out=msk[:], in_=masks_d[midx])

                opair = []
                for hc in range(4):
                    es = [[None] * 8, [None] * 8]
                    for kb in range(8):
                        ksrc = krot_p[hc] if kb < 4 else krot[hc]
                        kcol = 128 * (kb % 4)
                        qlo, qhi = SEGLO[kb], SEGHI[kb]
                        sw = qhi - qlo
                        rhs = q2g[hc] if kb < 4 else qrot[hc]
                        psA = P([128, 512], "simA", 2)
                        psB = P([128, 512], "simB", 2)
                        nc.tensor.matmul(psA[:, qlo:qhi],
                                         ksrc[0:64, kcol:kcol + 128],
                                         rhs[0:64, qlo:qhi],
                                         start=True, stop=True, tile_position=(0, 0))
                        nc.tensor.matmul(psB[:, qlo:qhi],
                                         ksrc[64:128, kcol:kcol + 128],
                                         rhs[64:128, qlo:qhi],
                                         start=True, stop=True, tile_position=(64, 0))
                        for hl, psx in ((0, psA), (1, psB)):
                            e = ap_.tile([128, 512], bf16, tag="es", bufs=16)
                            nc.vector.memset(e[:], 0.0)
                            nc.scalar.activation(out=e[:, qlo:qhi], in_=psx[:, qlo:qhi],
                                                 func=AF.Exp, bias=c_m30[:])
                            nc.vector.tensor_mul(e[:, qlo:qhi], e[:, qlo:qhi],
                                                 msk[:, SEGO[kb]:SEGO[kb] + sw])
                            es[hl][kb] = e

                    op = ap_.tile([128, 512], bf16, tag="opair", bufs=6)
                    for hl in range(2):
                        h = 2 * hc + hl
                        po = P([65, 512], "avo", 1)
                        for kb in range(8):
                            vsrc = va_p[kb % 4] if kb < 4 else va[kb % 4]
                            nc.tensor.matmul(po[:], vsrc[:, h, :], es[hl][kb][:],
                                             start=(kb == 0), stop=(kb == 7))
                        scr = ap_.tile([128, 512], f32, tag="scr", bufs=2)
                        nc.vector.reciprocal(out=scr[64:65, :], in_=po[64:65, :])
                        rbo = ap_.tile([64, 512], f32, tag="rbo", bufs=2)
                        bcast64(rbo[:], scr[64:65, :])
                        if hl == 0:
                            nc.vector.tensor_mul(op[0:64, :], po[0:64, :], rbo[:])
                        else:
                            ot = ap_.tile([64, 512], bf16, tag="otmp", bufs=3)
                            nc.vector.tensor_mul(ot[:], po[0:64, :], rbo[:])
                            nc.sync.dma_start(out=op[64:128, :], in_=ot[:])
                    opair.append(op)

                # --- wo + residual -> x2 ---
                for m in range(8):
                    ps = P([128, 512], "mm", 2)
                    for pc in range(4):
                        nc.tensor.matmul(ps[:], wo_t[pc][:, 128 * m:128 * m + 128],
                                         opair[pc][:],
                                         start=(pc == 0), stop=(pc == 3))
                    x2c = ap_.tile([128, 512], f32, tag="x2sb", bufs=3)
                    nc.vector.tensor_add(x2c[:], ps[:], xw[m][:, 1:513])
                    nc.sync.dma_start(
                        out=x2buf[128 * m:128 * m + 128,
                                  1 + W * (wi - 1):1 + W * wi],
                        in_=x2c[:])

            prev = (krot, va)

    # ---------------- feed-forward phase ----------------
    with (
        tc.tile_pool(name=f"ffw{l}", bufs=1) as fwp,
        tc.tile_pool(name=f"ff{l}", bufs=1) as fp_,
        tc.tile_pool(name=f"ffps{l}", bufs=1, space="PSUM") as pp2,
    ):
        w1a_t, w1g_t = [], []
        for cc in range(8):
            t = fwp.tile([128, HIDP], bf16, tag=f"w1a{cc}")
            nc.sync.dma_start(out=t[:], in_=w1a_d[l, 128 * cc:128 * cc + 128, :])
            w1a_t.append(t)
            t = fwp.tile([128, HIDP], bf16, tag=f"w1g{cc}")
            nc.sync.dma_start(out=t[:], in_=w1g_d[l, 128 * cc:128 * cc + 128, :])
            w1g_t.append(t)
        w2_t = []
        for fpc in range(FCH):
            t = fwp.tile([128, DIM], bf16, tag=f"w2{fpc}")
            nc.sync.dma_start(out=t[:], in_=w2_d[l, 128 * fpc:128 * fpc + 128, :])
            w2_t.append(t)
        gam = fwp.tile([128, 8], f32, tag="gam")
        nc.sync.dma_start(out=gam[:], in_=lng_d[l])
        bet = fwp.tile([128, 8], f32, tag="bet")
        nc.sync.dma_start(out=bet[:], in_=lnb_d[l])
        b1a_t = fwp.tile([128, FCH], f32, tag="b1a")
        nc.sync.dma_start(out=b1a_t[:], in_=b1a_d[l])
        b1g_t = fwp.tile([128, FCH], f32, tag="b1g")
        nc.sync.dma_start(out=b1g_t[:], in_=b1g_d[l])
        b2_t = fwp.tile([128, 8], f32, tag="b2")
        nc.sync.dma_start(out=b2_t[:], in_=b2_d[l])

        ff_lo, ff_hi = FFW[l]
        for wi in range(ff_lo, ff_hi):
            bx = 1 + W * (wi - 1)
            xf = []
            for cc in range(8):
                t = fp_.tile([128, 513], f32, tag="xf", bufs=8)
                nc.sync.dma_start(out=t[:],
                                  in_=x2buf[128 * cc:128 * cc + 128, bx - 1:bx + 512])
                xf.append(t)

            def sh(cc):
                o = 1 if cc < 4 else 0
                return xf[cc][:, o:o + 512]

            # LN stats over 1024 dims (partition reduce via DVE tree + gpsimd)
            acc = fp_.tile([128, 512], f32, tag="lnt", bufs=5)
            nc.vector.tensor_add(acc[:], sh(0), sh(1))
            for cc in range(2, 8):
                nc.vector.tensor_add(acc[:], acc[:], sh(cc))
            S = fp_.tile([128, 512], f32, tag="lnS", bufs=1)
            nc.gpsimd.partition_all_reduce(S[:], acc[:], channels=128,
                                           reduce_op=bass_isa.ReduceOp.add)
            acc2 = fp_.tile([128, 512], f32, tag="lnt", bufs=5)
            sqt = fp_.tile([128, 512], f32, tag="lnt", bufs=5)
            nc.vector.tensor_mul(sqt[:], sh(0), sh(0))
            nc.vector.tensor_copy(acc2[:], sqt[:])
            for cc in range(1, 8):
                nc.vector.tensor_mul(sqt[:], sh(cc), sh(cc))
                nc.vector.tensor_add(acc2[:], acc2[:], sqt[:])
            Q = fp_.tile([128, 512], f32, tag="lnQ", bufs=1)
            nc.gpsimd.partition_all_reduce(Q[:], acc2[:], channels=128,
                                           reduce_op=bass_isa.ReduceOp.add)
            v1 = fp_.tile([128, 512], f32, tag="lnt", bufs=5)
            nc.vector.scalar_tensor_tensor(out=v1[:], in0=S[:], scalar=1.0 / DIM,
                                           in1=S[:], op0=ALU.mult, op1=ALU.mult)
            v2 = fp_.tile([128, 512], f32, tag="lnt", bufs=5)
            nc.vector.tensor_sub(v2[:], Q[:], v1[:])
            sd = fp_.tile([128, 512], f32, tag="lnt", bufs=5)
            nc.scalar.activation(out=sd[:], in_=v2[:], func=AF.Sqrt,
                                 bias=c_eps5[:], scale=c_invd[:])
            rstd = fp_.tile([128, 512], f32, tag="rstd", bufs=1)
            nc.vector.reciprocal(out=rstd[:], in_=sd[:])

            hb = []
            for cc in range(8):
                t = fp_.tile([128, 512], f32, tag="lnt", bufs=5)
                nc.vector.scalar_tensor_tensor(out=t[:], in0=S[:], scalar=-1.0 / DIM,
                                               in1=sh(cc), op0=ALU.mult, op1=ALU.add)
                e = fp_.tile([128, 512], f32, tag="lnt", bufs=5)
                nc.vector.tensor_mul(e[:], t[:], rstd[:])
                hbc = fp_.tile([128, 512], bf16, tag="hb", bufs=8)
                nc.vector.tensor_scalar(out=hbc[:], in0=e[:],
                                        scalar1=gam[:, cc:cc + 1],
                                        scalar2=bet[:, cc:cc + 1],
                                        op0=ALU.mult, op1=ALU.add)
                hb.append(hbc)

            # F1: h1g[fp] = (Za + b1a) * gelu(Zg + b1g)
            h1g = []
            for fpc in range(FCH):
                za = pp2.tile([128, 512], f32, tag="mm", bufs=4)
                for cc in range(8):
                    nc.tensor.matmul(za[:], w1a_t[cc][:, 128 * fpc:128 * fpc + 128],
                                     hb[cc][:], start=(cc == 0), stop=(cc == 7))
                zg = pp2.tile([128, 512], f32, tag="mm", bufs=4)
                for cc in range(8):
                    nc.tensor.matmul(zg[:], w1g_t[cc][:, 128 * fpc:128 * fpc + 128],
                                     hb[cc][:], start=(cc == 0), stop=(cc == 7))
                gbf = fp_.tile([128, 512], bf16, tag="gbf", bufs=3)
                nc.scalar.activation(out=gbf[:], in_=zg[:], func=AF.Gelu,
                                     bias=b1g_t[:, fpc:fpc + 1])
                hg = fp_.tile([128, 512], bf16, tag="h1g", bufs=FCH + 1)
                nc.vector.scalar_tensor_tensor(out=hg[:], in0=za[:],
                                               scalar=b1a_t[:, fpc:fpc + 1],
                                               in1=gbf[:], op0=ALU.add, op1=ALU.mult)
                h1g.append(hg)

            # F2: x3 = x2 + b2 + h1g @ w2
            for m in range(8):
                ps = pp2.tile([128, 512], f32, tag="acc", bufs=3)
                for fpc in range(FCH):
                    nc.tensor.matmul(ps[:], w2_t[fpc][:, 128 * m:128 * m + 128],
                                     h1g[fpc][:], start=(fpc == 0), stop=(fpc == FCH - 1))
                x3c = fp_.tile([128, 512], f32, tag="x3", bufs=3)
                nc.vector.scalar_tensor_tensor(out=x3c[:], in0=ps[:],
                                               scalar=b2_t[:, m:m + 1],
                                               in1=xf[m][:, 1:513],
                                               op0=ALU.add, op1=ALU.add)
                if l == 0:
                    dst = xdst[128 * m:128 * m + 128, 1 + W * (wi - 1):1 + W * wi]
                else:
                    dst = out_d[128 * m:128 * m + 128, W * (wi - 2):W * (wi - 1)]
                nc.sync.dma_start(out=dst, in_=x3c[:])


# ---------------- host entry point ----------------

_NC = None


def _get_nc():
    global _NC
    if _NC is None:
        _NC = build_nc()
    return _NC


def kernel(x, wqkv, q_scale, k_scale, wo, ln_g, ln_b, w1, b1, w2, b2):
    x = np.asarray(x, np.float32)
    wqkv = np.asarray(wqkv, np.float32)
    q_scale = np.asarray(q_scale, np.float32)
    k_scale = np.asarray(k_scale, np.float32)
    wo = np.asarray(wo, np.float32)
    ln_g = np.asarray(ln_g, np.float32)
    ln_b = np.asarray(ln_b, np.float32)
    w1 = np.asarray(w1, np.float32)
    b1 = np.asarray(b1, np.float32)
    w2 = np.asarray(w2, np.float32)
    b2 = np.asarray(b2, np.float32)

    # q_scale/k_scale per head-dim, duplicated for rope tables
    # shared (all-core) tensors
    w1a = np.zeros((DEPTH, DIM, HIDP), np.float32)
    w1g = np.zeros((DEPTH, DIM, HIDP), np.float32)
    w1a[:, :, :HID] = w1[:, :, :HID]
    w1g[:, :, :HID] = w1[:, :, HID:]
    w2p = np.zeros((DEPTH, HIDP, DIM), np.float32)
    w2p[:, :HID, :] = w2
    b1a = np.zeros((DEPTH, HIDP), np.float32)
    b1g = np.zeros((DEPTH, HIDP), np.float32)
    b1a[:, :HID] = b1[:, :HID]
    b1g[:, :HID] = b1[:, HID:]

    def vec_chunks(v, nch):  # [DEPTH, nch*128] -> [DEPTH, 128, nch]
        return np.transpose(v.reshape(DEPTH, nch, 128), (0, 2, 1)).copy()

    shared = {
        "wqkv": wqkv.astype(bf),
        "wo": wo.astype(bf),
        "w1a": w1a.astype(bf),
        "w1g": w1g.astype(bf),
        "w2": w2p.astype(bf),
        "ones16": _ones16(),
        "gvec": _gvec(),
        "lng": vec_chunks(ln_g, 8),
        "lnb": vec_chunks(ln_b, 8),
        "b1a": vec_chunks(b1a, FCH),
        "b1g": vec_chunks(b1g, FCH),
        "b2": vec_chunks(b2, 8),
    }

    in_maps = []
    for c in range(NCORES):
        b, qt = c // 4, c % 4
        s = OWN * qt
        lo, hi = s - 1537, s + OWN
        xin = np.zeros((T0 + 1, DIM), np.float32)
        clo = max(lo, 0)
        xin[clo - lo:hi - lo] = x[b, clo:hi]
        tabs = _rope_tables(s, q_scale, k_scale)
        tabs_np = np.stack([np.stack(tl) for tl in tabs])  # [2, 4, 128, T0]
        m = dict(shared)
        m["xin"] = np.ascontiguousarray(xin.T)
        m["tabs"] = tabs_np
        m["masks"] = _masks(first_core=(qt == 0))
        in_maps.append(m)

    nc = _get_nc()
    res = run_bass_kernel_spmd(nc, in_maps, list(range(NCORES)))

    out = np.empty((B, N, DIM), np.float32)
    for c in range(NCORES):
        b, qt = c // 4, c % 4
        s = OWN * qt
        out[b, s:s + OWN] = res.results[c]["out"].T
    return out


# revision 7
# speedup vs baseline: 24.5190x; 24.5190x over previous
"""Trainium2 Bass kernel for nn_LocalTransformerBlock (sparse sliding-window attention).

Self-contained: accepts FULL inputs, shards across 8 NeuronCores internally
(batch x sequence-quarter, with redundant halo compute; no collectives),
returns the FULL output.

Layout strategy: activations flow TRANSPOSED ([feature, token]) so every
matmul keeps weights stationary (lhsT) and streams activation columns.
Attention is computed per 512-token window; window w attends windows
{w-1, w} with exact 513-token causal masking, per head-pair with row-packed
(tile_position) K=64 matmuls. Softmax skips max-subtraction (inputs are
bounded: |sim| <= 8 unmasked, <= ~98 masked; a -30 bias keeps exp in fp32
range) and masks multiplicatively AFTER exp with 0/1 bf16 masks.
"""

import numpy as np
import ml_dtypes

import concourse.bass as bass
import concourse.mybir as mybir
import concourse.tile as tile
import concourse.bass_isa as bass_isa
from concourse import bacc
from concourse.bass_utils import run_bass_kernel_spmd

f32 = mybir.dt.float32
bf16 = mybir.dt.bfloat16
AF = mybir.ActivationFunctionType
ALU = mybir.AluOpType
bf = ml_dtypes.bfloat16

# ---------------- problem constants (hardcoded) ----------------
B, N, DIM = 2, 8192, 1024
HEADS, DH, W = 8, 64, 512
INNER = 512
HID = 2730
HIDP = 2816          # padded to 22*128
FCH = HIDP // 128    # 22
DEPTH = 2
QK_SCALE = 8.0
NCORES = 8
OWN = 2048           # tokens owned per core
T0 = 3584            # L0 token span (7 windows: 3 halo + 4 own)
SRCW = [7, 6]        # qkv windows per layer
ATTW = [(1, 7), (1, 6)]   # attention windows [lo, hi)
FFW = [(1, 7), (2, 6)]    # feed-forward windows [lo, hi)
NMASK = 11           # 6 (L0) + 5 (L1) attention windows

# per-key-chunk active query segment [qlo, qhi)
SEGLO = [0, 0, 0, 0, 0, 128, 256, 384]
SEGHI = [128, 256, 384, 512, 512, 512, 512, 512]
SEGO = [0, 128, 384, 768, 1280, 1792, 2176, 2432]  # col offset in packed mask


# ---------------- host-side table builders ----------------

def _rope_tables(s, q_scale, k_scale):
    """CA/CB tables [DEPTH][128, T0] f32 for q and k, per-core (abs position s)."""
    j = np.arange(T0)
    n = (s - 1536 + j).astype(np.float64)         # absolute token index
    r = (j % W).astype(np.float64)                # window-relative index
    d = np.arange(DH)
    inv_freq = 1.0 / (10000.0 ** ((2 * (d % 32)) / DH))
    f = n[None, :] * inv_freq[:, None]            # [64, T0]
    cosf, sinf = np.cos(f), np.sin(f)
    base = ((2 * (d % 32)) + 0.4 * DH) / (1.4 * DH)  # [64]
    scq = base[:, None] ** (r[None, :] / 256.0)   # [64, T0]
    sck = base[:, None] ** (-r[None, :] / 256.0)
    sign = np.where(d < 32, -1.0, 1.0)
    sig = (d + 32) % DH                           # sigma(d)
    out = []
    for l in range(DEPTH):
        qs, ks = q_scale[l].astype(np.float64), k_scale[l].astype(np.float64)
        caq = QK_SCALE * qs[:, None] * cosf * scq
        cbq = QK_SCALE * sign[:, None] * qs[sig][:, None] * sinf * scq
        cak = ks[:, None] * cosf * sck
        cbk = sign[:, None] * ks[sig][:, None] * sinf * sck
        out.append([np.tile(t, (2, 1)).astype(np.float32) for t in (caq, cbq, cak, cbk)])
    return out  # [l][4][128, T0]


def _masks(first_core):
    """[NMASK, 128, 2560] bf16 packed per-kb 0/1 masks."""
    m = np.zeros((NMASK, 128, 2560), np.float32)
    # which mask index is the first real window (prev fully masked)?
    # L0: attn wi 1..6 -> idx wi-1 ; first real = wi 3 -> idx 2
    # L1: attn vj 1..5 -> idx 6+(vj-1); first real = vj 2 -> idx 7
    first_idx = {2, 7} if first_core else set()
    for mi in range(NMASK):
        for kb in range(8):
            seg = np.arange(SEGLO[kb], SEGHI[kb])
            cr = np.arange(128)
            cglob = 128 * kb + cr
            if kb < 4:
                allow = cglob[:, None] >= seg[None, :]
                if mi in first_idx:
                    allow = np.zeros_like(allow)
            else:
                allow = seg[None, :] >= (cglob[:, None] - W)
            m[mi, :, SEGO[kb]:SEGO[kb] + len(seg)] = allow
    return m.astype(bf)


def _gvec():
    d = np.arange(DH)
    base = ((2 * (d % 32)) + 0.4 * DH) / (1.4 * DH)
    g = (base ** 2.0).astype(np.float32)
    return np.tile(g, 2)[:, None]  # [128, 1]


def _ones16():
    """[8, 128, 16] bf16: sqsum selector. fc 0..3 -> q heads, 4..7 -> k heads."""
    o = np.zeros((8, 128, 16), np.float32)
    r = np.arange(128)
    for fc in range(8):
        col = 2 * (fc % 4) + r // 64 + 8 * (fc // 4)
        o[fc, r, col] = 1.0
    return o.astype(bf)


# ---------------- device program ----------------

def build_nc(reps=1):
    nc = bacc.Bacc("TRN2", target_bir_lowering=False, debug=False,
                   num_devices=NCORES)

    def din(name, shape, dt):
        return nc.dram_tensor(name, shape, dt, kind="ExternalInput").ap()

    xin = din("xin", [DIM, T0 + 1], f32)
    wqkv_d = din("wqkv", [DEPTH, DIM, 3 * INNER], bf16)
    wo_d = din("wo", [DEPTH, INNER, DIM], bf16)
    w1a_d = din("w1a", [DEPTH, DIM, HIDP], bf16)
    w1g_d = din("w1g", [DEPTH, DIM, HIDP], bf16)
    w2_d = din("w2", [DEPTH, HIDP, DIM], bf16)
    tabs_d = din("tabs", [DEPTH, 4, 128, T0], f32)
    masks_d = din("masks", [NMASK, 128, 2560], bf16)
    ones16_d = din("ones16", [8, 128, 16], bf16)
    gvec_d = din("gvec", [128, 1], f32)
    lng_d = din("lng", [DEPTH, 128, 8], f32)
    lnb_d = din("lnb", [DEPTH, 128, 8], f32)
    b1a_d = din("b1a", [DEPTH, 128, FCH], f32)
    b1g_d = din("b1g", [DEPTH, 128, FCH], f32)
    b2_d = din("b2", [DEPTH, 128, 8], f32)
    out_d = nc.dram_tensor("out", [DIM, OWN], f32, kind="ExternalOutput").ap()

    with tile.TileContext(nc) as tc:
        with (
            tc.tile_pool(name="const", bufs=1) as constp,
            tc.tile_pool(name="dram", bufs=1, space="DRAM") as dramp,
        ):
            x1buf = dramp.tile([DIM, 3073], f32, tag="x1")
            x2buf = dramp.tile([DIM, 3073], f32, tag="x2")

            gvec_t = constp.tile([128, 1], f32, tag="gvec")
            nc.sync.dma_start(out=gvec_t[:], in_=gvec_d[:])
            ones16_t = []
            for fc in range(8):
                t = constp.tile([128, 16], bf16, tag=f"o16_{fc}")
                nc.sync.dma_start(out=t[:], in_=ones16_d[fc])
                ones16_t.append(t)
            zcol_t = constp.tile([128, 1], f32, tag="zcol")
            nc.vector.memset(zcol_t[:], 0.0)
            c_eps24 = constp.tile([128, 1], f32, tag="c_eps24")
            nc.vector.memset(c_eps24[:], 1e-24)
            c_m30 = constp.tile([128, 1], f32, tag="c_m30")
            nc.vector.memset(c_m30[:], -30.0)
            c_eps5 = constp.tile([128, 1], f32, tag="c_eps5")
            nc.vector.memset(c_eps5[:], 1e-5)
            c_invd = constp.tile([128, 1], f32, tag="c_invd")
            nc.vector.memset(c_invd[:], 1.0 / DIM)
            consts = (c_eps24, c_m30, c_eps5, c_invd)

            for rep in range(reps):
              for l in range(DEPTH):
                xsrc = xin if l == 0 else x1buf
                xdst = x1buf if l == 0 else None
                _layer(nc, tc, l, xsrc, xdst, x2buf, out_d,
                       wqkv_d, wo_d, w1a_d, w1g_d, w2_d, tabs_d, masks_d,
                       ones16_t, gvec_t, zcol_t, consts,
                       lng_d, lnb_d, b1a_d, b1g_d, b2_d)

    nc.compile()
    return nc


def _layer(nc, tc, l, xsrc, xdst, x2buf, out_d,
           wqkv_d, wo_d, w1a_d, w1g_d, w2_d, tabs_d, masks_d,
           ones16_t, gvec_t, zcol_t, consts, lng_d, lnb_d, b1a_d, b1g_d, b2_d):
    c_eps24, c_m30, c_eps5, c_invd = consts
    srcw = SRCW[l]
    att_lo, att_hi = ATTW[l]
    ff_lo, ff_hi = FFW[l]
    tbl_off = 0 if l == 0 else 512

    # ---------------- attention phase ----------------
    with (
        tc.tile_pool(name=f"wts{l}", bufs=1) as wtp,
        tc.tile_pool(name=f"attn{l}", bufs=1) as ap_,
        tc.tile_pool(name=f"psum{l}", bufs=1, space="PSUM") as pp,
    ):
        # weights resident
        wqkv_t = []
        for cc in range(8):
            t = wtp.tile([128, 3 * INNER], bf16, tag=f"wqkv{cc}")
            nc.sync.dma_start(out=t[:], in_=wqkv_d[l, 128 * cc:128 * cc + 128, :])
            wqkv_t.append(t)
        wo_t = []
        for pc in range(4):
            t = wtp.tile([128, DIM], bf16, tag=f"wo{pc}")
            nc.sync.dma_start(out=t[:], in_=wo_d[l, 128 * pc:128 * pc + 128, :])
            wo_t.append(t)

        # zero the shift-surrogate column of the x2 buffer
        for cc in range(8):
            nc.sync.dma_start(out=x2buf[128 * cc:128 * cc + 128, 0:1], in_=zcol_t[:])

        _pn = [0]

        def P(shape, tag, bufs):
            _pn[0] += 1
            return pp.tile(shape, f32, tag=tag, bufs=bufs, name=f"ps_{l}_{tag}_{_pn[0]}")

        prev = None  # (krot, va) of previous window
        for wi in range(srcw):
            base = 1 + W * wi
            tb = tbl_off + W * wi

            # --- load x window (fp32 for residual, bf16 for matmul rhs) ---
            xw, xb = [], []
            for cc in range(8):
                t = ap_.tile([128, 513], f32, tag="xw", bufs=8)
                nc.sync.dma_start(
                    out=t[:], in_=xsrc[128 * cc:128 * cc + 128, base - 1:base + 512])
                xw.append(t)
                tb16 = ap_.tile([128, 513], bf16, tag="xb", bufs=8)
                nc.vector.tensor_copy(tb16[:], t[:])
                xb.append(tb16)

            # --- rope tables for this window ---
            tabs = []
            for k in range(4):
                t = ap_.tile([128, 512], f32, tag="tab", bufs=5)
                nc.sync.dma_start(out=t[:], in_=tabs_d[l, k, :, tb:tb + 512])
                tabs.append(t)
            caq, cbq, cak, cbk = tabs

            # --- q/k (transposed out) + squares ---
            qraw, q2 = [], []
            ss_ps = P([16, 512], "ssacc", 1)
            for fc in range(8):
                ps = P([128, 512], "mm", 2)
                for cc in range(8):
                    sh = 1 if cc < 4 else 0
                    nc.tensor.matmul(
                        ps[:], wqkv_t[cc][:, 128 * fc:128 * fc + 128],
                        xb[cc][:, sh:sh + 512],
                        start=(cc == 0), stop=(cc == 7))
                raw = ap_.tile([128, 512], f32, tag="qraw", bufs=8)
                nc.scalar.copy(out=raw[:], in_=ps[:])
                sq = ap_.tile([128, 512], bf16, tag="q2", bufs=8)
                nc.scalar.activation(out=sq[:], in_=ps[:], func=AF.Square)
                qraw.append(raw)
                q2.append(sq)
                nc.tensor.matmul(ss_ps[:], ones16_t[fc][:], sq[:],
                                 start=(fc == 0), stop=(fc == 7))

            # --- v (natural orientation) + ones-augmented tiles ---
            va = []
            for tc_ in range(4):
                ps = P([128, 512], "mm", 2)
                for cc in range(8):
                    sh = 1 if cc < 4 else 0
                    nc.tensor.matmul(
                        ps[:], xb[cc][:, sh + 128 * tc_:sh + 128 * tc_ + 128],
                        wqkv_t[cc][:, 2 * INNER:3 * INNER],
                        start=(cc == 0), stop=(cc == 7))
                t = ap_.tile([128, 8, 65], bf16, tag="va", bufs=8)
                nc.vector.memset(t[:], 1.0)
                nc.scalar.copy(out=t[:, :, 0:64], in_=ps[:].rearrange("p (h d) -> p h d", h=8))
                va.append(t)

            # --- l2norm reciprocals ---
            ssq = ap_.tile([16, 512], f32, tag="ssq", bufs=3)
            nc.scalar.activation(out=ssq[:], in_=ss_ps[:], func=AF.Sqrt, bias=c_eps24[:16, :])
            rsq = ap_.tile([16, 512], f32, tag="rsq", bufs=3)
            nc.vector.reciprocal(out=rsq[:], in_=ssq[:])

            def bcast64(dst_ap, src_row_ap):
                src = bass.AP(tensor=src_row_ap.tensor, offset=src_row_ap.offset,
                              ap=[list(src_row_ap.ap[0]), [0, 64]] + list(src_row_ap.ap[1:]))
                nc.sync.dma_start(out=dst_ap, in_=src)

            # --- rotary + normalize (q gets xpos-recentering g on a copy) ---
            qrot, q2g, krot = [], [], []
            for fc in range(8):
                isq = fc < 4
                hc = fc % 4
                rb = ap_.tile([128, 512], f32, tag="rb", bufs=4)
                row0 = 2 * hc + (0 if isq else 8)
                bcast64(rb[0:64, :], rsq[row0:row0 + 1, :])
                bcast64(rb[64:128, :], rsq[row0 + 1:row0 + 2, :])

                rot = ap_.tile([128, 512], f32, tag="rot", bufs=4)
                for blk in range(4):
                    nc.sync.dma_start(
                        out=rot[32 * blk:32 * blk + 32, :],
                        in_=qraw[fc][32 * (blk ^ 1):32 * (blk ^ 1) + 32, :])

                ca, cb = (caq, cbq) if isq else (cak, cbk)
                t1 = ap_.tile([128, 512], f32, tag="rt", bufs=4)
                nc.vector.tensor_mul(t1[:], qraw[fc][:], ca[:])
                t2 = ap_.tile([128, 512], f32, tag="rt", bufs=4)
                nc.vector.tensor_mul(t2[:], rot[:], cb[:])
                t3 = ap_.tile([128, 512], f32, tag="rt", bufs=4)
                nc.vector.tensor_add(t3[:], t1[:], t2[:])
                rot_bf = ap_.tile([128, 512], bf16, tag="qk_bf", bufs=14)
                nc.vector.tensor_mul(rot_bf[:], t3[:], rb[:])
                if isq:
                    qrot.append(rot_bf)
                    t4 = ap_.tile([128, 512], bf16, tag="q2g", bufs=5)
                    nc.vector.tensor_scalar_mul(t4[:], rot_bf[:], gvec_t[:])
                    q2g.append(t4)
                else:
                    krot.append(rot_bf)

            # --- attention ---
            if wi >= att_lo and prev is not None:
                krot_p, va_p = prev
                midx = (wi - 1) if l == 0 else (6 + wi - 1)
                msk = ap_.tile([128, 2560], bf16, tag="msk", bufs=2)
                nc.sync.dma_start(out=msk[:], in_=masks_d[midx])

                opair = []
                for hc in range(4):
                    es = [[None] * 8, [None] * 8]
                    for kb in range(8):
                        ksrc = krot_p[hc] if kb < 4 else krot[hc]
                        kcol = 128 * (kb % 4)
                        qlo, qhi = SEGLO[kb], SEGHI[kb]
                        sw = qhi - qlo
                        rhs = q2g[hc] if kb < 4 else qrot[hc]
                        psA = P([128, 512], "simA", 2)
                        psB = P([128, 512], "simB", 2)
                        nc.tensor.matmul(psA[:, qlo:qhi],
                                         ksrc[0:64, kcol:kcol + 128],
                                         rhs[0:64, qlo:qhi],
                                         start=True, stop=True, tile_position=(0, 0))
                        nc.tensor.matmul(psB[:, qlo:qhi],
                                         ksrc[64:128, kcol:kcol + 128],
                                         rhs[64:128, qlo:qhi],
                                         start=True, stop=True, tile_position=(64, 0))
                        for hl, psx in ((0, psA), (1, psB)):
                            e = ap_.tile([128, 512], bf16, tag="es", bufs=16)
                            nc.vector.memset(e[:], 0.0)
                            nc.scalar.activation(out=e[:, qlo:qhi], in_=psx[:, qlo:qhi],
                                                 func=AF.Exp, bias=c_m30[:])
                            nc.vector.tensor_mul(e[:, qlo:qhi], e[:, qlo:qhi],
                                                 msk[:, SEGO[kb]:SEGO[kb] + sw])
                            es[hl][kb] = e

                    op = ap_.tile([128, 512], bf16, tag="opair", bufs=6)
                    for hl in range(2):
                        h = 2 * hc + hl
                        po = P([65, 512], "avo", 1)
                        for kb in range(8):
                            vsrc = va_p[kb % 4] if kb < 4 else va[kb % 4]
                            nc.tensor.matmul(po[:], vsrc[:, h, :], es[hl][kb][:],
                                             start=(kb == 0), stop=(kb == 7))
                        scr = ap_.tile([128, 512], f32, tag="scr", bufs=2)
                        nc.vector.reciprocal(out=scr[64:65, :], in_=po[64:65, :])
                        rbo = ap_.tile([64, 512], f32, tag="rbo", bufs=2)
                        bcast64(rbo[:], scr[64:65, :])
                        if hl == 0:
                            nc.vector.tensor_mul(op[0:64, :], po[0:64, :], rbo[:])
                        else:
                            ot = ap_.tile([64, 512], bf16, tag="otmp", bufs=3)
                            nc.vector.tensor_mul(ot[:], po[0:64, :], rbo[:])
                            nc.sync.dma_start(out=op[64:128, :], in_=ot[:])
                    opair.append(op)

                # --- wo + residual -> x2 ---
                for m in range(8):
                    ps = P([128, 512], "mm", 2)
                    for pc in range(4):
                        nc.tensor.matmul(ps[:], wo_t[pc][:, 128 * m:128 * m + 128],
                                         opair[pc][:],
                                         start=(pc == 0), stop=(pc == 3))
                    x2c = ap_.tile([128, 512], f32, tag="x2sb", bufs=3)
                    nc.vector.tensor_add(x2c[:], ps[:], xw[m][:, 1:513])
                    nc.sync.dma_start(
                        out=x2buf[128 * m:128 * m + 128,
                                  1 + W * (wi - 1):1 + W * wi],
                        in_=x2c[:])

            prev = (krot, va)

    # ---------------- feed-forward phase ----------------
    with (
        tc.tile_pool(name=f"ffw{l}", bufs=1) as fwp,
        tc.tile_pool(name=f"ff{l}", bufs=1) as fp_,
        tc.tile_pool(name=f"ffps{l}", bufs=1, space="PSUM") as pp2,
    ):
        w1a_t, w1g_t = [], []
        for cc in range(8):
            t = fwp.tile([128, HIDP], bf16, tag=f"w1a{cc}")
            nc.sync.dma_start(out=t[:], in_=w1a_d[l, 128 * cc:128 * cc + 128, :])
            w1a_t.append(t)
            t = fwp.tile([128, HIDP], bf16, tag=f"w1g{cc}")
            nc.sync.dma_start(out=t[:], in_=w1g_d[l, 128 * cc:128 * cc + 128, :])
            w1g_t.append(t)
        w2_t = []
        for fpc in range(FCH):
            t = fwp.tile([128, DIM], bf16, tag=f"w2{fpc}")
            nc.sync.dma_start(out=t[:], in_=w2_d[l, 128 * fpc:128 * fpc + 128, :])
            w2_t.append(t)
        gam = fwp.tile([128, 8], f32, tag="gam")
        nc.sync.dma_start(out=gam[:], in_=lng_d[l])
        bet = fwp.tile([128, 8], f32, tag="bet")
        nc.sync.dma_start(out=bet[:], in_=lnb_d[l])
        b1a_t = fwp.tile([128, FCH], f32, tag="b1a")
        nc.sync.dma_start(out=b1a_t[:], in_=b1a_d[l])
        b1g_t = fwp.tile([128, FCH], f32, tag="b1g")
        nc.sync.dma_start(out=b1g_t[:], in_=b1g_d[l])
        b2_t = fwp.tile([128, 8], f32, tag="b2")
        nc.sync.dma_start(out=b2_t[:], in_=b2_d[l])

        ff_lo, ff_hi = FFW[l]
        for wi in range(ff_lo, ff_hi):
            bx = 1 + W * (wi - 1)
            xf = []
            for cc in range(8):
                t = fp_.tile([128, 513], f32, tag="xf", bufs=8)
                nc.sync.dma_start(out=t[:],
                                  in_=x2buf[128 * cc:128 * cc + 128, bx - 1:bx + 512])
                xf.append(t)

            def sh(cc):
                o = 1 if cc < 4 else 0
                return xf[cc][:, o:o + 512]

            # LN stats over 1024 dims (partition reduce via DVE tree + gpsimd)
            acc = fp_.tile([128, 512], f32, tag="lnt", bufs=5)
            nc.vector.tensor_add(acc[:], sh(0), sh(1))
            for cc in range(2, 8):
                nc.vector.tensor_add(acc[:], acc[:], sh(cc))
            S = fp_.tile([128, 512], f32, tag="lnS", bufs=1)
            nc.gpsimd.partition_all_reduce(S[:], acc[:], channels=128,
                                           reduce_op=bass_isa.ReduceOp.add)
            acc2 = fp_.tile([128, 512], f32, tag="lnt", bufs=5)
            sqt = fp_.tile([128, 512], f32, tag="lnt", bufs=5)
            nc.vector.tensor_mul(sqt[:], sh(0), sh(0))
            nc.vector.tensor_copy(acc2[:], sqt[:])
            for cc in range(1, 8):
                nc.vector.tensor_mul(sqt[:], sh(cc), sh(cc))
                nc.vector.tensor_add(acc2[:], acc2[:], sqt[:])
            Q = fp_.tile([128, 512], f32, tag="lnQ", bufs=1)
            nc.gpsimd.partition_all_reduce(Q[:], acc2[:], channels=128,
                                           reduce_op=bass_isa.ReduceOp.add)
            v1 = fp_.tile([128, 512], f32, tag="lnt", bufs=5)
            nc.vector.scalar_tensor_tensor(out=v1[:], in0=S[:], scalar=1.0 / DIM,
                                           in1=S[:], op0=ALU.mult, op1=ALU.mult)
            v2 = fp_.tile([128, 512], f32, tag="lnt", bufs=5)
            nc.vector.tensor_sub(v2[:], Q[:], v1[:])
            sd = fp_.tile([128, 512], f32, tag="lnt", bufs=5)
            nc.scalar.activation(out=sd[:], in_=v2[:], func=AF.Sqrt,
                                 bias=c_eps5[:], scale=c_invd[:])
            rstd = fp_.tile([128, 512], f32, tag="rstd", bufs=1)
            nc.vector.reciprocal(out=rstd[:], in_=sd[:])

            hb = []
            for cc in range(8):
                t = fp_.tile([128, 512], f32, tag="lnt", bufs=5)
                nc.vector.scalar_tensor_tensor(out=t[:], in0=S[:], scalar=-1.0 / DIM,
                                               in1=sh(cc), op0=ALU.mult, op1=ALU.add)
                e = fp_.tile([128, 512], f32, tag="lnt", bufs=5)
                nc.vector.tensor_mul(e[:], t[:], rstd[:])
                hbc = fp_.tile([128, 512], bf16, tag="hb", bufs=8)
                nc.vector.tensor_scalar(out=hbc[:], in0=e[:],
                                        scalar1=gam[:, cc:cc + 1],
                                        scalar2=bet[:, cc:cc + 1],
                                        op0=ALU.mult, op1=ALU.add)
                hb.append(hbc)

            # F1: h1g[fp] = (Za + b1a) * gelu(Zg + b1g)
            h1g = []
            for fpc in range(FCH):
                za = pp2.tile([128, 512], f32, tag="mm", bufs=4)
                for cc in range(8):
                    nc.tensor.matmul(za[:], w1a_t[cc][:, 128 * fpc:128 * fpc + 128],
                                     hb[cc][:], start=(cc == 0), stop=(cc == 7))
                zg = pp2.tile([128, 512], f32, tag="mm", bufs=4)
                for cc in range(8):
                    nc.tensor.matmul(zg[:], w1g_t[cc][:, 128 * fpc:128 * fpc + 128],
                                     hb[cc][:], start=(cc == 0), stop=(cc == 7))
                gbf = fp_.tile([128, 512], bf16, tag="gbf", bufs=3)
                nc.scalar.activation(out=gbf[:], in_=zg[:], func=AF.Gelu,
                                     bias=b1g_t[:, fpc:fpc + 1])
                hg = fp_.tile([128, 512], bf16, tag="h1g", bufs=FCH + 1)
                nc.vector.scalar_tensor_tensor(out=hg[:], in0=za[:],
                                               scalar=b1a_t[:, fpc:fpc + 1],
                                               in1=gbf[:], op0=ALU.add, op1=ALU.mult)
                h1g.append(hg)

            # F2: x3 = x2 + b2 + h1g @ w2
            for m in range(8):
                ps = pp2.tile([128, 512], f32, tag="acc", bufs=3)
                for fpc in range(FCH):
                    nc.tensor.matmul(ps[:], w2_t[fpc][:, 128 * m:128 * m + 128],
                                     h1g[fpc][:], start=(fpc == 0), stop=(fpc == FCH - 1))
                x3c = fp_.tile([128, 512], f32, tag="x3", bufs=3)
                nc.vector.scalar_tensor_tensor(out=x3c[:], in0=ps[:],
                                               scalar=b2_t[:, m:m + 1],
                                               in1=xf[m][:, 1:513],
                                               op0=ALU.add, op1=ALU.add)
                if l == 0:
                    dst = xdst[128 * m:128 * m + 128, 1 + W * (wi - 1):1 + W * wi]
                else:
                    dst = out_d[128 * m:128 * m + 128, W * (wi - 2):W * (wi - 1)]
                nc.sync.dma_start(out=dst, in_=x3c[:])


# ---------------- host entry point ----------------

_NC = None


def _get_nc():
    global _NC
    if _NC is None:
        _NC = build_nc()
    return _NC


def kernel(x, wqkv, q_scale, k_scale, wo, ln_g, ln_b, w1, b1, w2, b2):
    x = np.asarray(x, np.float32)
    wqkv = np.asarray(wqkv, np.float32)
    q_scale = np.asarray(q_scale, np.float32)
    k_scale = np.asarray(k_scale, np.float32)
    wo = np.asarray(wo, np.float32)
    ln_g = np.asarray(ln_g, np.float32)
    ln_b = np.asarray(ln_b, np.float32)
    w1 = np.asarray(w1, np.float32)
    b1 = np.asarray(b1, np.float32)
    w2 = np.asarray(w2, np.float32)
    b2 = np.asarray(b2, np.float32)

    # q_scale/k_scale per head-dim, duplicated for rope tables
    # shared (all-core) tensors
    w1a = np.zeros((DEPTH, DIM, HIDP), np.float32)
    w1g = np.zeros((DEPTH, DIM, HIDP), np.float32)
    w1a[:, :, :HID] = w1[:, :, :HID]
    w1g[:, :, :HID] = w1[:, :, HID:]
    w2p = np.zeros((DEPTH, HIDP, DIM), np.float32)
    w2p[:, :HID, :] = w2
    b1a = np.zeros((DEPTH, HIDP), np.float32)
    b1g = np.zeros((DEPTH, HIDP), np.float32)
    b1a[:, :HID] = b1[:, :HID]
    b1g[:, :HID] = b1[:, HID:]

    def vec_chunks(v, nch):  # [DEPTH, nch*128] -> [DEPTH, 128, nch]
        return np.transpose(v.reshape(DEPTH, nch, 128), (0, 2, 1)).copy()

    shared = {
        "wqkv": wqkv.astype(bf),
        "wo": wo.astype(bf),
        "w1a": w1a.astype(bf),
        "w1g": w1g.astype(bf),
        "w2": w2p.astype(bf),
        "ones16": _ones16(),
        "gvec": _gvec(),
        "lng": vec_chunks(ln_g, 8),
        "lnb": vec_chunks(ln_b, 8),
        "b1a": vec_chunks(b1a, FCH),
        "b1g": vec_chunks(b1g, FCH),
        "b2": vec_chunks(b2, 8),
    }

    in_maps = []
    for c in range(NCORES):
        b, qt = c // 4, c % 4
        s = OWN * qt
        lo, hi = s - 1537, s + OWN
        xin = np.zeros((T0 + 1, DIM), np.float32)
        clo = max(lo, 0)
        xin[clo - lo:hi - lo] = x[b, clo:hi]
        tabs = _rope_tables(s, q_scale, k_scale)
        tabs_np = np.stack([np.stack(tl) for tl in tabs])  # [2, 4, 128, T0]
        m = dict(shared)
        m["xin"] = np.ascontiguousarray(xin.T)
        m["tabs"] = tabs_np
        m["masks"] = _masks(first_core=(qt == 0))
        in_maps.append(m)

    nc = _get_nc()
    res = run_bass_kernel_spmd(nc, in_maps, list(range(NCORES)))

    out = np.empty((B, N, DIM), np.float32)
    for c in range(NCORES):
        b, qt = c // 4, c % 4
        s = OWN * qt
        out[b, s:s + OWN] = res.results[c]["out"].T
    return out
